# revision 13
# baseline (speedup 1.0000x reference)
"""Trainium2 Bass kernel for nn_BulkHamiltonian.

Math (derived from the reference, verified numerically):
  For each Bloch wavevector k = (kx, ky):
    phase1 = sqrt(3)*kx              ; K1 = exp(i*phase1)
    phase2 = sqrt(3)/2*kx + 1.5*ky   ; K2 = exp(i*phase2)
  With r11+r22+r33 = 1.5*I, M^-1 = [[0,I],[I,0]] (row swap), the output
  H[b] (8x8 complex64) is mostly CONSTANT:
    H[0:4,0:4] = 0       H[0:4,4:8] = I4
    H[4:6,0:2] = 1.5*I2  H[6:8,2:4] = 1.5*I2
    H[4:8,4:8] = L12 (constant, +-0.2i pattern)
    H[4:6,2:4] = -A_tr = -P + iQ     H[6:8,0:2] = -A_bl = -P - iQ
  where with c1=cos(phase1), s1=sin(phase1), c2=cos(phase2), s2=sin(phase2):
    P00 = 0.75 + 0.75*c1       Q00 = 0.75*s1
    P01 = P10 = (sqrt3/4)*(1 - c1)   Q01 = Q10 = -(sqrt3/4)*s1
    P11 = 0.25 + 0.25*c1 + c2  Q11 = 0.25*s1 + s2
  So per batch element only 16 of the 128 output floats vary; the other
  112 floats are a fixed template (mostly zeros).

Kernel strategy (pure data parallel over 8 cores, 125000 elems each):
  - Output viewed as [N, 128] float32 (complex64 interleaved re/im).
  - Persistent SBUF out-buffers [128 partitions, NB, 128] hold the
    constant template (initialized once); each tile iteration rewrites
    only the 16 varying columns, then DMAs the whole [128, NB*128]
    block to HBM (contiguous 512B per element -> near-peak DMA).
  - sin/cos via ScalarE Sin activation with free affine
    (cos(x) = sin(x + pi/2)); the rest are tiny DVE affine ops.
"""

import sys
import types

import numpy as np

import concourse.bacc as bacc
import concourse.bass as bass
import concourse.mybir as mybir
from concourse import bass_utils
from concourse.tile import TileContext


def _ensure_axon_hooks():
    """bass_utils imports antenv.axon_hooks when tracing is requested (e.g.
    BASS_TRACE=1); that module isn't shipped in this image. Provide it,
    backed by the boot helper's ctypes NTFF hook when available."""
    try:
        import antenv.axon_hooks  # noqa: F401
        return
    except ImportError:
        pass
    hook = None
    try:
        from trn_agent_boot.trn_boot import _ntff_profile_via_ctypes

        hook = _ntff_profile_via_ctypes("/opt/axon/libaxon_pjrt.so")
    except Exception:
        hook = None
    mod = types.ModuleType("antenv.axon_hooks")
    mod.get_axon_ntff_profile_hook = lambda: hook
    mod.set_axon_ntff_profile_hook = lambda h: None
    try:
        import antenv

        sys.modules["antenv.axon_hooks"] = mod
        antenv.axon_hooks = mod
    except ImportError:
        sys.modules["antenv.axon_hooks"] = mod


_ensure_axon_hooks()

B_TOTAL = 1_000_000
N_CORES = 8
N_PER_CORE = B_TOTAL // N_CORES  # 125000
NB = 64                          # batch elements per partition per tile
N_OBUF = 3                       # output template buffers (pipeline depth)

F32 = mybir.dt.float32

SQ3 = 1.7320508075688772
ISQ3 = 0.5773502691896258        # 1/sqrt(3)
C34 = 0.4330127018922193         # sqrt(3)/4
PI = 3.141592653589793
PIO2 = 1.5707963267948966
TWOPI = 6.283185307179586
INV2PI = 0.15915494309189535
MAGIC = 12582912.0               # 1.5 * 2**23: float32 round-to-nearest trick

# Cody-Waite split of 2*pi into three float32 constants (c1 + c2 + c3 ~ 2pi,
# products k*c1, k*c2 exact for small integer k)
CW1 = float(np.float32(6.28125))
_r = TWOPI - float(np.float32(6.28125))
_c2bits = np.float32(_r).view(np.uint32) & np.uint32(0xFFFFF000)
CW2 = float(_c2bits.view(np.float32))
CW3 = float(np.float32(_r - float(_c2bits.view(np.float32))))

# float-column (within the 128-float row of H) -> constant value
CONST_COLS = [
    (8, 1.0), (26, 1.0), (44, 1.0), (62, 1.0),       # I4 in H[0:4,4:8]
    (64, 1.5), (82, 1.5), (100, 1.5), (118, 1.5),    # 1.5*I2 blocks
    (75, 0.2), (89, -0.2), (111, 0.2), (125, -0.2),  # L12 block
]


def _tiles(n, nb):
    """Tile descriptors (start_row, nb_t) covering [0, n).

    Full tiles of 128*nb rows; if a remainder exists, one final tile is
    emitted whose range overlaps the previous tile (identical data is
    written twice, which is harmless).
    """
    chunk = 128 * nb
    out = []
    full = n // chunk
    for i in range(full):
        out.append((i * chunk, nb))
    rem = n - full * chunk
    if rem:
        nb_t = (rem + 127) // 128
        start = n - 128 * nb_t
        assert start >= 0, "n must be >= 128*ceil(rem/128)"
        out.append((start, nb_t))
    return out


def build_nc(n=N_PER_CORE, nb=NB, enable_asserts=False):
    nc = bacc.Bacc(
        "TRN2",
        target_bir_lowering=False,
        debug=False,
        enable_asserts=enable_asserts,
    )
    k_ap = nc.dram_tensor("k_in", [n, 2], F32, kind="ExternalInput").ap()
    o_ap = nc.dram_tensor("h_out", [n, 128], F32, kind="ExternalOutput").ap()

    obufs = [
        nc.alloc_sbuf_tensor(f"obuf{i}", [128, nb, 128], F32).ap()
        for i in range(N_OBUF)
    ]
    A = mybir.AluOpType
    AF = mybir.ActivationFunctionType

    def init_buf(ob):
        # Zero-fill split across vector+gpsimd so the two halves run in
        # parallel; interleaved with the first tiles so tile 0's columns
        # don't queue behind every buffer's init.
        half = nb // 2
        nc.vector.memset(ob[:, :half, :], 0.0)
        nc.gpsimd.memset(ob[:, half:, :], 0.0)
        for idx, (col, val) in enumerate(CONST_COLS):
            eng = nc.vector if idx % 2 == 0 else nc.gpsimd
            eng.memset(ob[:, :, col], val)

    with TileContext(nc) as tc:
        with tc.tile_pool(name="work", bufs=2) as pool:
            for t, (start, nbt) in enumerate(_tiles(n, nb)):
                if t < N_OBUF:
                    init_buf(obufs[t])
                o = obufs[t % N_OBUF]
                rows = 128 * nbt
                dma_eng = nc.sync if t % 2 == 0 else nc.scalar

                kt = pool.tile([128, nbt, 2], F32, tag="kt", bufs=3, name="kt")
                dma_eng.dma_start(
                    kt,
                    k_ap[start:start + rows].rearrange("(p n) c -> p n c", p=128),
                )
                kx = kt[:, :, 0]
                ky = kt[:, :, 1]

                def tile_(tag):
                    return pool.tile([128, nbt], F32, tag=tag, name=tag)

                c1 = tile_("c1"); s1 = tile_("s1"); c2 = tile_("c2"); s2 = tile_("s2")
                v = tile_("v"); w2 = tile_("w2"); w3 = tile_("w3")
                x1 = tile_("x1"); t1 = tile_("t1"); q1 = tile_("q1")
                y1 = tile_("y1"); yc1 = tile_("yc1")
                x2 = tile_("x2"); t2 = tile_("t2"); q2 = tile_("q2")
                y2 = tile_("y2"); yc2 = tile_("yc2")

                # phase1 = sqrt3*kx; range-reduce into [-pi, pi] via
                # round(x/2pi) (magic-number trick) + Cody-Waite cascade.
                nc.vector.tensor_scalar(x1, kx, SQ3, None, A.mult)
                nc.vector.tensor_scalar(t1, x1, INV2PI, MAGIC, A.mult, A.add)
                nc.vector.tensor_scalar(q1, t1, MAGIC, None, A.subtract)
                nc.vector.cody_waite_cascade(y1, x1, q1, CW1, CW2, CW3)
                nc.vector.add_range_wrap(yc1, y1, PIO2, PI, TWOPI)

                # phase2 = 1.5*(kx/sqrt3 + ky)
                nc.vector.scalar_tensor_tensor(v, kx, ISQ3, ky, A.mult, A.add)
                nc.vector.tensor_scalar(x2, v, 1.5, None, A.mult)
                nc.vector.tensor_scalar(t2, x2, INV2PI, MAGIC, A.mult, A.add)
                nc.vector.tensor_scalar(q2, t2, MAGIC, None, A.subtract)
                nc.vector.cody_waite_cascade(y2, x2, q2, CW1, CW2, CW3)
                nc.vector.add_range_wrap(yc2, y2, PIO2, PI, TWOPI)

                nc.scalar.activation(s1, y1, AF.Sin)
                nc.scalar.activation(c1, yc1, AF.Sin)
                nc.scalar.activation(s2, y2, AF.Sin)
                nc.scalar.activation(c2, yc2, AF.Sin)

                # helpers: w3 = -0.25*c1 - 0.25, w2 = 0.25*s1
                nc.vector.tensor_scalar(w3, c1, -0.25, -0.25, A.mult, A.add)
                nc.vector.tensor_scalar(w2, s1, 0.25, None, A.mult)

                # ---- real parts ----
                # -P00 = -0.75 - 0.75*c1  at cols 68, 96
                nc.scalar.activation(o[:, :nbt, 68], c1, AF.Copy, bias=-0.75, scale=-0.75)
                nc.scalar.activation(o[:, :nbt, 96], c1, AF.Copy, bias=-0.75, scale=-0.75)
                # -P01 = C34*c1 - C34  at cols 70, 84, 98, 112
                nc.vector.tensor_scalar(o[:, :nbt, 70], c1, C34, -C34, A.mult, A.add)
                nc.vector.tensor_scalar(o[:, :nbt, 84], c1, C34, -C34, A.mult, A.add)
                nc.vector.tensor_scalar(o[:, :nbt, 98], c1, C34, -C34, A.mult, A.add)
                nc.vector.tensor_scalar(o[:, :nbt, 112], c1, C34, -C34, A.mult, A.add)
                # -P11 = w3 - c2  at cols 86, 114
                nc.vector.tensor_sub(o[:, :nbt, 86], w3, c2)
                nc.vector.tensor_sub(o[:, :nbt, 114], w3, c2)

                # ---- imag parts ----
                # +Q00 = 0.75*s1 at col 69 ; -Q00 at col 97
                nc.scalar.activation(o[:, :nbt, 69], s1, AF.Copy, bias=0.0, scale=0.75)
                nc.scalar.activation(o[:, :nbt, 97], s1, AF.Copy, bias=0.0, scale=-0.75)
                # +Q01 = -C34*s1 at cols 71, 85 ; -Q01 = +C34*s1 at cols 99, 113
                nc.vector.tensor_scalar(o[:, :nbt, 71], s1, -C34, None, A.mult)
                nc.vector.tensor_scalar(o[:, :nbt, 85], s1, -C34, None, A.mult)
                nc.vector.tensor_scalar(o[:, :nbt, 99], s1, C34, None, A.mult)
                nc.vector.tensor_scalar(o[:, :nbt, 113], s1, C34, None, A.mult)
                # +Q11 = w2 + s2 at col 87 ; -Q11 = -w2 - s2 at col 115
                nc.vector.tensor_add(o[:, :nbt, 87], w2, s2)
                nc.vector.scalar_tensor_tensor(o[:, :nbt, 115], w2, -1.0, s2, A.mult, A.subtract)

                dma_eng.dma_start(
                    o_ap[start:start + rows].rearrange("(p n) c -> p n c", p=128),
                    o[:, :nbt, :],
                )
    nc.compile()
    return nc


_CACHE = {}


def _get_nc():
    if "nc" not in _CACHE:
        _CACHE["nc"] = build_nc()
    return _CACHE["nc"]


def run_spmd(k_flat, **kwargs):
    """k_flat: [B_TOTAL, 2] float32. Returns (out [B_TOTAL, 128] f32, results obj)."""
    shards = np.ascontiguousarray(k_flat).reshape(N_CORES, N_PER_CORE, 2)
    nc = _get_nc()
    in_maps = [{"k_in": shards[i]} for i in range(N_CORES)]
    res = bass_utils.run_bass_kernel_spmd(
        nc, in_maps, core_ids=list(range(N_CORES)), **kwargs
    )
    out = np.empty((B_TOTAL, 128), dtype=np.float32)
    for i in range(N_CORES):
        out[i * N_PER_CORE:(i + 1) * N_PER_CORE] = res.results[i]["h_out"]
    return out, res


def kernel(k):
    k = np.asarray(k, dtype=np.float32).reshape(B_TOTAL, 2)
    out, _ = run_spmd(k)
    return out.view(np.complex64).reshape(B_TOTAL, 8, 8)
